# revision 1
# baseline (speedup 1.0000x reference)
"""Trainium2 Bass kernel: 3-layer GCN + classifier (nn_AdvancedGCN).

Strategy (8 NeuronCores, node-sharded graph parallel):
  - Host: partition nodes by id range across 8 cores. All graph structure is
    precomputed on host (index math only): degree normalization dinv, and a
    "degree-round" slot table that turns the edge segment-sum into dense
    prefix adds. Because dma_gather indices are int16, the all-gathered
    feature table is addressed in two halves (rows < 32768 and the rest);
    each core keeps two node orderings (sorted by lo-/hi-indegree) so each
    half's rounds cover a contiguous slot prefix.
  - Device, per GCN layer: per-block PE transpose + matmul, fused dinv row
    scale (norm = dinv[src]*dinv[dst] factorizes), AllGather of scaled
    features u into a shared DRAM table, chunked dma_gather of edge sources +
    DVE prefix adds into two accumulators, a small gather-unpermute to merge
    them, then fused BN(scale/shift) + ReLU + residual.
  - Classifier MLP + log_softmax on the local shard; host inverse-permutes.
"""

import sys

if "/opt/trn_rl_repo" not in sys.path:
    sys.path.insert(0, "/opt/trn_rl_repo")

import numpy as np

N = 50000
E = 800000
D_IN = 128
HID = [128, 64, 32]
C_OUT = 2
EPS = 1e-5
NCORES = 8
P = 128
NLOC = N // NCORES                  # 6250 real nodes per core
NLOCP = ((NLOC + P) // P) * P       # 6272 padded (always > NLOC: zero rows)
NTAB = NCORES * NLOCP
CH_BLOCKS = 32                      # gather chunk size (blocks of 128 slots)
NSWQ = 4                            # SWDGE queues (gathers round-robin)
MINW = 64                           # min table row width (256B f32 stride rule)


# --------------------------------------------------------------------------
# Host-side graph preprocessing (pure index math)
# --------------------------------------------------------------------------

def _wrap16(flat):
    """[M] -> [128, M/16] int16 in dma_gather's wrapped+replicated layout."""
    m = len(flat)
    assert m % 16 == 0
    a = flat.reshape(m // 16, 16).T
    return np.ascontiguousarray(np.tile(a, (8, 1)).astype(np.int16))


def prep_structure(edge_index, n, ncores, nloc, nlocp, ch_blocks):
    """Returns (dinv[n], ordersA, per_core list of dicts(idxw, permw),
    chunks, totb, split). chunks: list of (half, idx_off, blocks, pieces)
    where pieces = [(off_blocks, nbk)]. The table is split at a core-aligned
    row (split = split_core*nlocp) so a source's half is decided by its core
    id alone; each core keeps two node orderings (by lo-/hi-indegree)."""
    src = edge_index[0].astype(np.int64)
    dst = edge_index[1].astype(np.int64)
    ntab = ncores * nlocp
    assert nlocp > nloc, "need zero pad rows"
    split_core = max(1, min(ncores - 1, 32768 // nlocp))
    split = split_core * nlocp
    assert (ntab - split) - (nlocp - nloc) < 32768
    deg = np.bincount(dst, minlength=n).astype(np.float32) + np.float32(1.0)
    dinv = (np.float32(1.0) / np.sqrt(deg)).astype(np.float32)

    core = dst // nloc
    is_lo_src = (src // nloc) < split_core

    zlo = nloc                       # core 0 zero-pad row (< split)
    zhi_rel = nloc                   # core split_core zero-pad row - split

    core_data = []
    roundsL = []
    roundsH = []
    for c in range(ncores):
        lo = c * nloc
        m = core == c
        d_loc = dst[m] - lo
        isl = is_lo_src[m]
        cl = np.bincount(d_loc[isl], minlength=nloc)
        chh = np.bincount(d_loc[~isl], minlength=nloc)
        orderA = np.argsort(-cl, kind="stable")
        posA = np.empty(nloc, np.int64)
        posA[orderA] = np.arange(nloc)
        orderB = np.argsort(-chh, kind="stable")
        posB = np.empty(nloc, np.int64)
        posB[orderB] = np.arange(nloc)
        core_data.append((d_loc, m, isl, posA, posB, orderA, orderB, cl, chh))
        roundsL.append(np.array([(cl > k).sum()
                                 for k in range(int(cl.max()) if nloc else 0)],
                                np.int64))
        roundsH.append(np.array([(chh > k).sum()
                                 for k in range(int(chh.max()) if nloc else 0)],
                                np.int64))

    # global node -> table row (uses each core's posA)
    pos_all = np.concatenate([cd[3] for cd in core_data])
    gslot = (np.arange(n) // nloc) * nlocp + pos_all

    RL = max(len(r) for r in roundsL)
    RH = max(len(r) for r in roundsH)

    def common_nbk(rlist, R):
        out = []
        for k in range(R):
            mx = max(int(r[k]) if k < len(r) else 0 for r in rlist)
            out.append(max(1, (mx + P - 1) // P))
        return out

    nbkL = common_nbk(roundsL, RL)
    nbkH = common_nbk(roundsH, RH)

    # rounds laid out consecutively (hi half first), then uniform chunks of
    # <= ch_blocks blocks per half; rounds may split across chunk boundaries.
    # pieces = (goff_blocks, accoff_blocks, nblocks)
    roundoff = {}
    cum_blocks = 0
    half_ranges = {}
    for half, nbks in ((1, nbkH), (0, nbkL)):
        start = cum_blocks
        for k, nbk in enumerate(nbks):
            roundoff[(half, k)] = cum_blocks
            cum_blocks += nbk
        half_ranges[half] = (start, cum_blocks)
    totb = cum_blocks
    chunks = []
    for half in (1, 0):
        s, e = half_ranges[half]
        nbks = nbkH if half == 1 else nbkL
        b0 = s
        while b0 < e:
            cb = min(ch_blocks, e - b0)
            pieces = []
            for k, nbk in enumerate(nbks):
                ro = roundoff[(half, k)]
                lo_ = max(ro, b0)
                hi_ = min(ro + nbk, b0 + cb)
                if lo_ < hi_:
                    pieces.append((lo_ - b0, lo_ - ro, hi_ - lo_))
            chunks.append((half, b0 * P, cb, tuple(pieces)))
            b0 += cb

    per_core = []
    ordersA = []
    for c in range(ncores):
        d_loc, m, isl, posA, posB, orderA, orderB, cl, chh = core_data[c]
        ordersA.append(orderA)
        g_src = gslot[src[m]]
        flat = np.empty(totb * P, np.int64)
        for half, nbks in ((0, nbkL), (1, nbkH)):
            pad = zlo if half == 0 else zhi_rel
            for k, nbk in enumerate(nbks):
                o = roundoff[(half, k)] * P
                flat[o:o + nbk * P] = pad
        for half in (0, 1):
            mh = isl if half == 0 else ~isl
            dd = d_loc[mh]
            ss = g_src[mh] - (0 if half == 0 else split)
            ldst = (posA if half == 0 else posB)[dd]
            # sort by (dst slot, src row): round k takes the k-th smallest
            # src of each dst -> per-round sources cluster by quantile band
            o = np.lexsort((ss, ldst))
            ldst = ldst[o]
            ss = ss[o]
            cnts = np.bincount(ldst, minlength=nloc)
            starts = np.concatenate([[0], np.cumsum(cnts)[:-1]])
            kk = np.arange(len(ldst)) - starts[ldst]
            nbks = nbkL if half == 0 else nbkH
            ro = np.array([roundoff[(half, k)] for k in range(len(nbks))],
                          np.int64)
            flat[ro[kk] * P + ldst] = ss
        assert flat.max() < 32768 and flat.min() >= 0, (flat.min(), flat.max())
        idxw = _wrap16(flat)
        permf = np.full(nlocp, nloc, np.int64)
        permf[:nloc] = posB[orderA]
        permw = _wrap16(permf)
        per_core.append({"idxw": idxw, "permw": permw})

    return dinv, ordersA, per_core, chunks, totb, split


# --------------------------------------------------------------------------
# Device graph builder
# --------------------------------------------------------------------------

def build_graph(nc, *, ncores, nloc, nlocp, ntab, split, d_in, hid, c_out,
                chunks, totb, debug_taps=False, repeat=1):
    import concourse.bass as bass
    import concourse.mybir as mybir
    import concourse.tile as tile
    from concourse.masks import make_identity

    f32 = mybir.dt.float32
    bf16 = mybir.dt.bfloat16
    i16 = mybir.dt.int16
    Alu = mybir.AluOpType
    Act = mybir.ActivationFunctionType
    nb = nlocp // P
    dims = [d_in] + list(hid)
    nlayer = len(hid)
    tdts = [bf16 if l == 0 else f32 for l in range(nlayer)]
    # gather-table row widths: row bytes must be a multiple of 256
    dws = [max(hid[l], 128 if tdts[l] == bf16 else MINW)
           for l in range(nlayer)]

    # ---------------- declare I/O ----------------
    x_t = nc.dram_tensor("x", [nlocp, d_in], f32, kind="ExternalInput")
    dinv_t = nc.dram_tensor("dinv", [nlocp, 1], f32, kind="ExternalInput")
    idxs_t = nc.dram_tensor("idxs", [P, totb * 8], i16, kind="ExternalInput")
    perm_t = nc.dram_tensor("perm", [P, nlocp // 16], i16,
                            kind="ExternalInput")
    wt = {}
    for l in range(nlayer):
        di, do = dims[l], dims[l + 1]
        wt[f"W{l}"] = nc.dram_tensor(f"W{l}", [di, do], f32,
                                     kind="ExternalInput")
        for nm in (f"b{l}", f"bn{l}_g", f"bn{l}_b", f"bn{l}_m", f"bn{l}_v"):
            wt[nm] = nc.dram_tensor(nm, [do], f32, kind="ExternalInput")
        if di != do:
            wt[f"skip{l}_W"] = nc.dram_tensor(f"skip{l}_W", [di, do], f32,
                                              kind="ExternalInput")
            wt[f"skip{l}_b"] = nc.dram_tensor(f"skip{l}_b", [do], f32,
                                              kind="ExternalInput")
    h_last = hid[-1]
    h_mid = h_last // 2
    wt["cls1_W"] = nc.dram_tensor("cls1_W", [h_last, h_mid], f32,
                                  kind="ExternalInput")
    wt["cls1_b"] = nc.dram_tensor("cls1_b", [h_mid], f32,
                                  kind="ExternalInput")
    wt["cls2_W"] = nc.dram_tensor("cls2_W", [h_mid, c_out], f32,
                                  kind="ExternalInput")
    wt["cls2_b"] = nc.dram_tensor("cls2_b", [c_out], f32,
                                  kind="ExternalInput")
    out_t = nc.dram_tensor("out", [nlocp, c_out], f32, kind="ExternalOutput")
    dbg = {}
    if debug_taps:
        for l in range(nlayer):
            do = dims[l + 1]
            for nm in ("u", "aggA", "aggB", "agg", "h"):
                dbg[f"{nm}{l}"] = nc.dram_tensor(
                    f"dbg_{nm}{l}", [nlocp, do], f32, kind="ExternalOutput")

    with tile.TileContext(nc) as tc:
        with tc.tile_pool(name="cpool", bufs=1) as cp, \
             tc.tile_pool(name="wpool", bufs=1) as wp, \
             tc.tile_pool(name="ppool", bufs=2, space="PSUM") as pp, \
             tc.tile_pool(name="dpool", bufs=1, space="DRAM") as dp:

            # ------------- constants -------------
            ident = cp.tile([P, P], f32, tag="ident", name="ident")
            make_identity(nc, ident[:])
            ones_row = cp.tile([1, P], f32, tag="ones", name="ones")
            nc.vector.memset(ones_row[:], 1.0)
            eps_sb = cp.tile([1, 1], f32, tag="eps", name="eps")
            nc.vector.memset(eps_sb[:], float(EPS))

            idx_sb = cp.tile([P, totb * 8], i16, tag="idx", name="idx")
            nc.sync.dma_start(idx_sb[:], idxs_t.ap())
            perm_sb = cp.tile([P, nlocp // 16], i16, tag="perm", name="perm")
            nc.sync.dma_start(perm_sb[:], perm_t.ap())
            dinv_sb = cp.tile([P, nb, 1], f32, tag="dinv", name="dinv")
            nc.sync.dma_start(
                dinv_sb[:], dinv_t.ap().rearrange("(j p) d -> p j d", p=P))

            def load_row(name, width):
                t = cp.tile([1, width], f32, tag=f"row_{name}",
                            name=f"row_{name}")
                nc.sync.dma_start(t[:], wt[name].ap()[None, :])
                return t

            def bcast(row_ap, width, tag):
                """[1,width] SBUF -> [P,width] SBUF via PE rank-1 matmul."""
                ps = pp.tile([P, width], f32, tag="bc_ps", name="bc_ps")
                nc.tensor.matmul(ps[:], ones_row[:, :], row_ap,
                                 start=True, stop=True)
                rep = cp.tile([P, width], f32, tag=tag, name=tag)
                nc.vector.tensor_copy(rep[:], ps[:])
                return rep

            W_sb = {}
            s_rep = {}
            t_rep = {}
            skipW_sb = {}
            skipb_rep = {}
            for l in range(nlayer):
                di, do = dims[l], dims[l + 1]
                W_sb[l] = wp.tile([di, do], f32, tag=f"W{l}", name=f"W{l}")
                nc.sync.dma_start(W_sb[l][:], wt[f"W{l}"].ap())
                g_r = load_row(f"bn{l}_g", do)
                be_r = load_row(f"bn{l}_b", do)
                m_r = load_row(f"bn{l}_m", do)
                v_r = load_row(f"bn{l}_v", do)
                b_r = load_row(f"b{l}", do)
                sq = cp.tile([1, do], f32, tag=f"sq{l}", name=f"sq{l}")
                nc.scalar.activation(sq[:], v_r[:], Act.Sqrt,
                                     bias=eps_sb[0:1, 0:1])
                rc = cp.tile([1, do], f32, tag=f"rc{l}", name=f"rc{l}")
                nc.vector.reciprocal(rc[:], sq[:])
                s_row = cp.tile([1, do], f32, tag=f"s{l}", name=f"s{l}")
                nc.vector.tensor_tensor(s_row[:], rc[:], g_r[:], op=Alu.mult)
                t_row = cp.tile([1, do], f32, tag=f"t{l}", name=f"t{l}")
                nc.vector.tensor_tensor(t_row[:], b_r[:], m_r[:],
                                        op=Alu.subtract)
                nc.vector.tensor_tensor(t_row[:], t_row[:], s_row[:],
                                        op=Alu.mult)
                nc.vector.tensor_tensor(t_row[:], t_row[:], be_r[:],
                                        op=Alu.add)
                s_rep[l] = bcast(s_row[:], do, f"srep{l}")
                t_rep[l] = bcast(t_row[:], do, f"trep{l}")
                if di != do:
                    skipW_sb[l] = wp.tile([di, do], f32, tag=f"sW{l}",
                                          name=f"sW{l}")
                    nc.sync.dma_start(skipW_sb[l][:], wt[f"skip{l}_W"].ap())
                    sb_r = load_row(f"skip{l}_b", do)
                    skipb_rep[l] = bcast(sb_r[:], do, f"sbrep{l}")
            cls1W_sb = wp.tile([h_last, h_mid], f32, tag="c1W", name="c1W")
            nc.sync.dma_start(cls1W_sb[:], wt["cls1_W"].ap())
            cls2W_sb = wp.tile([h_mid, c_out], f32, tag="c2W", name="c2W")
            nc.sync.dma_start(cls2W_sb[:], wt["cls2_W"].ap())
            cls1b_rep = bcast(load_row("cls1_b", h_mid)[:], h_mid, "c1brep")
            cls2b_rep = bcast(load_row("cls2_b", c_out)[:], c_out, "c2brep")

            def _net():
                # ------------- input activations -------------
                hin = wp.tile([P, nb, d_in], f32, tag="h0", name="h0")
                nc.sync.dma_start(
                    hin[:], x_t.ap().rearrange("(j p) d -> p j d", p=P))
                res = hin  # layer-0 skip is identity

                def transpose_blocks(src_tile, di, tag):
                    """[P, nb, di] -> [di, nb, P] via PE transpose."""
                    ht = wp.tile([P, nb, P], f32, tag=tag, name=tag)
                    for b in range(nb):
                        pt = pp.tile([P, P], f32, tag="pt", name="pt")
                        nc.tensor.transpose(pt[:di, :], src_tile[:, b, :di],
                                            ident[:])
                        nc.vector.tensor_copy(ht[:di, b, :], pt[:di, :])
                    return ht

                for l in range(nlayer):
                    di, do = dims[l], dims[l + 1]
                    dw = dws[l]
                    tdt = tdts[l]   # gather-table dtype
                    ht = transpose_blocks(hin, di, "ht")

                    # u = dinv * (h @ W), written into a dw-wide (zero-padded)
                    # tile that becomes the gather table row.
                    u_sb = wp.tile([P, nb, dw], tdt, tag="u", name="u")
                    if dw > do:
                        nc.vector.memset(u_sb[:], 0.0)
                    for b in range(nb):
                        pu = pp.tile([P, do], f32, tag="pu", name="pu")
                        nc.tensor.matmul(pu[:], ht[:di, b, :], W_sb[l][:, :],
                                         start=True, stop=True)
                        nc.vector.tensor_tensor(
                            u_sb[:, b, :do], pu[:],
                            dinv_sb[:, b, :].to_broadcast([P, do]),
                            op=Alu.mult)

                    # residual for this layer
                    if di != do:
                        res_new = wp.tile([P, nb, do], f32, tag="res", name="res")
                        for b in range(nb):
                            pr = pp.tile([P, do], f32, tag="pr", name="pr")
                            nc.tensor.matmul(pr[:], ht[:di, b, :],
                                             skipW_sb[l][:, :],
                                             start=True, stop=True)
                            nc.vector.tensor_tensor(res_new[:, b, :], pr[:],
                                                    skipb_rep[l][:, :do],
                                                    op=Alu.add)
                        res = res_new

                    # AllGather u -> u_all table
                    u_bounce = dp.tile([nlocp, dw], tdt, tag=f"ub{l}",
                                       name=f"ub{l}")
                    nc.sync.dma_start(
                        u_bounce[:].rearrange("(j p) d -> p j d", p=P), u_sb[:])
                    u_all = dp.tile([ntab, dw], tdt,
                                    addr_space="Shared" if ncores > 4 else "Local",
                                    tag=f"ua{l}", name=f"ua{l}")
                    if ncores == 1:
                        # timeline/profiling variant: no collective; fill the
                        # whole table so gathers read real (finite) data
                        for _c in range(ntab // nlocp):
                            nc.sync.dma_start(
                                u_all[_c * nlocp:(_c + 1) * nlocp, :],
                                u_bounce[:])
                    else:
                        nc.gpsimd.collective_compute(
                            "AllGather", Alu.bypass,
                            replica_groups=[list(range(ncores))],
                            ins=[u_bounce.opt()], outs=[u_all.opt()])

                    # two accumulators: A (lo half, ordering A = canonical),
                    # B (hi half, ordering B)
                    aggA = wp.tile([P, nb, do], f32, tag="aggA", name="aggA")
                    aggB = wp.tile([P, nb, do], f32, tag="aggB", name="aggB")
                    nc.vector.tensor_copy(aggA[:], u_sb[:, :, :do])  # self term
                    nc.vector.memset(aggB[:], 0.0)

                    ch_max = max([1] + [b for (_h, _o, b, _p) in chunks])
                    nhi = sum(1 for (_h, _o, _b, _p) in chunks if _h == 1)
                    nq = getattr(nc, "num_swdge_queues", 1)

                    def do_unpermute():
                        # merge B into A via gather-unpermute through DRAM.
                        # DVE cast to bf16 + HWDGE store (keeps SWDGE queues
                        # free); the perm gather is split across all queues.
                        bdw = max(do, 128)
                        bb_sb = wp.tile([P, nb, bdw], bf16, tag="ht",
                                        name="bb_sb")
                        nc.vector.tensor_copy(bb_sb[:, :, :do], aggB[:])
                        bounceB = dp.tile([nlocp, bdw], bf16, tag=f"bb{l}",
                                          name=f"bb{l}")
                        nc.sync.dma_start(
                            bounceB[:, :].rearrange("(j p) d -> p j d", p=P),
                            bb_sb[:])
                        gp = wp.tile([P, nb, bdw], bf16, tag="aggB", name="gp")
                        b0 = 0
                        for qi in range(nq):
                            nbq = (nb - b0 + (nq - 1 - qi)) // (nq - qi)
                            if nbq == 0:
                                continue
                            nc.gpsimd.dma_gather(
                                out_ap=gp[:, b0:b0 + nbq, :],
                                in_ap=bounceB[:, :],
                                idxs_ap=perm_sb[:, b0 * 8:(b0 + nbq) * 8],
                                num_idxs=nbq * P,
                                num_idxs_reg=nbq * P,
                                elem_size=bdw,
                                single_packet=False,
                                queue_num=qi)
                            b0 += nbq
                        nc.vector.tensor_tensor(aggA[:], aggA[:],
                                                gp[:, :, :do], op=Alu.add)

                    for ci, (half, ioff, cblk, pieces) in enumerate(chunks):
                        g = wp.tile([P, ch_max, dw], tdt,
                                    tag=f"g{ci % 4}", name=f"g{ci % 4}")
                        src_ap = u_all[:, :] if half == 0 else u_all[split:, :]
                        nc.gpsimd.dma_gather(
                            out_ap=g[:, :cblk, :],
                            in_ap=src_ap,
                            idxs_ap=idx_sb[:, ioff // 16:(ioff + cblk * P) // 16],
                            num_idxs=cblk * P,
                            num_idxs_reg=cblk * P,
                            elem_size=dw,
                            single_packet=False,
                            queue_num=ci % nq)
                        tgt = aggA if half == 0 else aggB
                        for goff, aoff, nbk in pieces:
                            nc.vector.tensor_tensor(
                                tgt[:, aoff:aoff + nbk, :do],
                                tgt[:, aoff:aoff + nbk, :do],
                                g[:, goff:goff + nbk, :do], op=Alu.add)
                        if ci == nhi - 1:
                            do_unpermute()

                    if debug_taps:
                        nc.gpsimd.dma_start(
                            dbg[f"u{l}"].ap().rearrange("(j p) d -> p j d", p=P),
                            u_sb[:, :, :do])
                        nc.sync.dma_start(
                            dbg[f"aggA{l}"].ap().rearrange("(j p) d -> p j d", p=P),
                            aggA[:])
                        nc.sync.dma_start(
                            dbg[f"aggB{l}"].ap().rearrange("(j p) d -> p j d", p=P),
                            aggB[:])

                    if debug_taps:
                        nc.sync.dma_start(
                            dbg[f"agg{l}"].ap().rearrange("(j p) d -> p j d", p=P),
                            aggA[:])

                    # post: h = relu((agg*dinv)*s + t) + res
                    scr = wp.tile([P, nb, do], f32, tag="aggB", name="scr")
                    nc.vector.tensor_tensor(
                        scr[:], aggA[:],
                        dinv_sb[:, :, :].to_broadcast([P, nb, do]),
                        op=Alu.mult)
                    nc.vector.tensor_tensor(
                        aggA[:], scr[:],
                        s_rep[l][:, None, :].to_broadcast([P, nb, do]),
                        op=Alu.mult)
                    nc.vector.tensor_tensor(
                        scr[:], aggA[:],
                        t_rep[l][:, None, :].to_broadcast([P, nb, do]),
                        op=Alu.add)
                    nc.scalar.activation(aggA[:], scr[:], Act.Relu)
                    hout = wp.tile([P, nb, do], f32, tag=f"h{(l + 1) % 2}",
                                   name=f"h{(l + 1) % 2}")
                    nc.vector.tensor_tensor(hout[:], aggA[:], res[:], op=Alu.add)
                    if debug_taps:
                        nc.sync.dma_start(
                            dbg[f"h{l}"].ap().rearrange("(j p) d -> p j d", p=P),
                            hout[:])
                    hin = hout
                    res = hout

                # ------------- classifier -------------
                ht3 = transpose_blocks(hin, h_last, "ht")
                c1 = wp.tile([P, nb, h_mid], f32, tag="u", name="c1")
                for b in range(nb):
                    pc = pp.tile([P, h_mid], f32, tag="pu", name="pc")
                    nc.tensor.matmul(pc[:], ht3[:h_last, b, :], cls1W_sb[:, :],
                                     start=True, stop=True)
                    nc.vector.tensor_tensor(c1[:, b, :], pc[:],
                                            cls1b_rep[:, :], op=Alu.add)
                nc.scalar.activation(c1[:], c1[:], Act.Relu)
                c1t = transpose_blocks(c1, h_mid, "ht")
                logits = wp.tile([P, nb, c_out], f32, tag="logits", name="logits")
                for b in range(nb):
                    pc2 = pp.tile([P, c_out], f32, tag="pu", name="pc2")
                    nc.tensor.matmul(pc2[:], c1t[:h_mid, b, :], cls2W_sb[:, :],
                                     start=True, stop=True)
                    nc.vector.tensor_tensor(logits[:, b, :], pc2[:],
                                            cls2b_rep[:, :], op=Alu.add)
                # log_softmax along last axis
                mx = wp.tile([P, nb, 1], f32, tag="mx", name="mx")
                nc.vector.tensor_reduce(mx[:], logits[:],
                                        axis=mybir.AxisListType.X, op=Alu.max)
                nc.vector.tensor_tensor(logits[:], logits[:],
                                        mx[:].to_broadcast([P, nb, c_out]),
                                        op=Alu.subtract)
                ex = wp.tile([P, nb, c_out], f32, tag="ex", name="ex")
                nc.scalar.activation(ex[:], logits[:], Act.Exp)
                sm = wp.tile([P, nb, 1], f32, tag="sm", name="sm")
                nc.vector.tensor_reduce(sm[:], ex[:], axis=mybir.AxisListType.X,
                                        op=Alu.add)
                nc.scalar.activation(sm[:], sm[:], Act.Ln)
                nc.vector.tensor_tensor(logits[:], logits[:],
                                        sm[:].to_broadcast([P, nb, c_out]),
                                        op=Alu.subtract)
                nc.sync.dma_start(
                    out_t.ap().rearrange("(j p) d -> p j d", p=P), logits[:])

            for _rep in range(repeat):
                _net()

    return nc


# --------------------------------------------------------------------------
# Top-level entry: full inputs -> full output
# --------------------------------------------------------------------------

_CACHE = {}
RUN_KW = {}          # extra kwargs for run_bass_kernel_spmd (e.g. trace=True)
LAST_RESULTS = None  # BassKernelResults of the last run (for profiling)


def kernel(**inputs):
    import concourse.bacc as bacc
    from concourse.bass_utils import run_bass_kernel_spmd

    x = np.asarray(inputs["x"], np.float32)
    edge_index = np.asarray(inputs["edge_index"])

    dinv, ordersA, per_core, chunks, totb, split = prep_structure(
        edge_index, N, NCORES, NLOC, NLOCP, CH_BLOCKS)

    key = ("g", totb, tuple((h, o, b) for h, o, b, _ in chunks))
    if key not in _CACHE:
        nc = bacc.Bacc("TRN2", target_bir_lowering=False, debug=False,
                       num_devices=NCORES, num_swdge_queues=NSWQ)
        build_graph(nc, ncores=NCORES, nloc=NLOC, nlocp=NLOCP, ntab=NTAB,
                    split=split, d_in=D_IN, hid=HID, c_out=C_OUT,
                    chunks=chunks, totb=totb)
        nc.compile()
        _CACHE[key] = nc
    nc = _CACHE[key]

    wnames = []
    dims = [D_IN] + HID
    for l in range(len(HID)):
        wnames += [f"W{l}", f"b{l}", f"bn{l}_g", f"bn{l}_b", f"bn{l}_m",
                   f"bn{l}_v"]
        if dims[l] != dims[l + 1]:
            wnames += [f"skip{l}_W", f"skip{l}_b"]
    wnames += ["cls1_W", "cls1_b", "cls2_W", "cls2_b"]

    in_maps = []
    for c in range(NCORES):
        lo = c * NLOC
        order = ordersA[c]
        xp = np.zeros((NLOCP, D_IN), np.float32)
        xp[:NLOC] = x[lo:lo + NLOC][order]
        # pad rows get dinv=0 so their u rows (the gather-pad targets) are 0
        dl = np.zeros((NLOCP, 1), np.float32)
        dl[:NLOC, 0] = dinv[lo:lo + NLOC][order]
        m = {"x": xp, "dinv": dl, "idxs": per_core[c]["idxw"],
             "perm": per_core[c]["permw"]}
        for nm in wnames:
            m[nm] = np.ascontiguousarray(np.asarray(inputs[nm], np.float32))
        in_maps.append(m)

    try:
        results = run_bass_kernel_spmd(nc, in_maps,
                                       core_ids=list(range(NCORES)), **RUN_KW)
    except Exception:
        # transient axon-worker hiccups: retry once
        import time as _time
        _time.sleep(5)
        results = run_bass_kernel_spmd(nc, in_maps,
                                       core_ids=list(range(NCORES)), **RUN_KW)
    global LAST_RESULTS
    LAST_RESULTS = results
    out = np.empty((N, C_OUT), np.float32)
    for c in range(NCORES):
        lo = c * NLOC
        out[lo + ordersA[c]] = results.results[c]["out"][:NLOC]
    return out

